# revision 5
# baseline (speedup 1.0000x reference)
"""Dense2DSpatialTransformer (bilinear warp, N(0,1) flow) on 8 TRN2 cores.

V3 design (vs baseline): the per-pixel 2D gather is still realized as
horizontal one-hot selects over 8 column shifts on 9 candidate rows,
followed by two vertical one-hot selects — but every select element now
carries TWO fp16 pixels packed in one int32:

  * Phase 0 builds, in DRAM, an fp32 replicate-padded image (for the exact
    sparse fixup) and an int32 "HP" image whose element c is the packed
    fp16 pair (I[h,c], I[h,c+1]).  A single 8-way one-hot select on HP
    yields BOTH bilinear corners (west+east) at once, halving the
    copy_predicated element traffic that dominates the runtime.
  * Masks and fractional weights are computed from an fp16 copy of the
    flow with tensor_scalar ops (4x DVE mode for 2-byte dtypes).
  * The dense path is fp16 end-to-end (tolerance is 2e-2 of max; fp16
    error here is ~3e-3 absolute).  Output is fp16, widened on host.
  * Outliers (|disp| beyond the 8-wide window, ~1e-4 of pixels) are
    computed exactly in fp32 on device via per-element indirect-DMA
    gathers, as in the baseline, and scattered over the dense result.
"""
import sys

for _p in ("/opt/trn_rl_repo", "/opt/trn_rl_repo/concourse",
           "/root/.axon_site/_ro/trn_rl_repo"):
    if _p not in sys.path:
        sys.path.insert(0, _p)

import numpy as np

import concourse.bass as bass
import concourse.bacc as bacc
import concourse.mybir as mybir
import concourse.tile as tile
from concourse.bass import IndirectOffsetOnAxis
from concourse.bass_utils import run_bass_kernel_spmd

f32 = np.float32
FP = mybir.dt.float32
FP16 = mybir.dt.float16
I32 = mybir.dt.int32
I16 = mybir.dt.int16

B, H, W = 16, 1024, 1024
NCORES = 8
BPC = B // NCORES           # images per core
PAD = 8
PP = H + 2 * PAD            # padded image side (1040)
S_LO, S_HI = -4, 3          # dense integer-shift window (per axis)
F = 512                     # free-dim tile width
NROW = H // 128             # row blocks per image
NCOL = W // F               # col chunks per image
HW = H * W
OUT_TAIL = 128              # scratch tail for fixup padding writes
INIT = -1                   # shift covered by the one-hot init copy

AL = mybir.AluOpType


def _build_program(nout):
    nc = bacc.Bacc("TRN2", target_bir_lowering=False, debug=False,
                   enable_asserts=False, num_devices=NCORES)

    img_d = nc.dram_tensor("img", [BPC, H, W], FP, kind="ExternalInput")
    flow_d = nc.dram_tensor("flow", [BPC * 2 * HW], FP, kind="ExternalInput")
    opos_d = nc.dram_tensor("opos", [nout], I32, kind="ExternalInput")
    odh_d = nc.dram_tensor("odh", [nout], I32, kind="ExternalInput")
    odw_d = nc.dram_tensor("odw", [nout], I32, kind="ExternalInput")
    oh_d = nc.dram_tensor("oh", [nout], FP, kind="ExternalInput")
    oh1_d = nc.dram_tensor("oh1", [nout], FP, kind="ExternalInput")
    ow_d = nc.dram_tensor("ow", [nout], FP, kind="ExternalInput")
    ow1_d = nc.dram_tensor("ow1", [nout], FP, kind="ExternalInput")
    obase_d = nc.dram_tensor("obase", [nout], FP, kind="ExternalInput")
    ppad_d = nc.dram_tensor("ppad", [BPC * PP * PP], FP, kind="Internal")
    hp_d = nc.dram_tensor("hp", [BPC, PP, PP], I32, kind="Internal")
    out_d = nc.dram_tensor("out", [BPC * HW + OUT_TAIL], FP16,
                           kind="ExternalOutput")

    img = img_d.ap()
    flowf = flow_d.ap()
    flow4 = flowf.rearrange("(b c h w) -> b c h w", b=BPC, c=2, h=H, w=W)
    ppf = ppad_d.ap()
    pp3 = ppf.rearrange("(b h w) -> b h w", b=BPC, h=PP, w=PP)
    hp3 = hp_d.ap()
    outf = out_d.ap()
    out3 = outf[0:BPC * HW].rearrange("(b h w) -> b h w", b=BPC, h=H, w=W)

    v = nc.vector
    g = nc.gpsimd

    with tile.TileContext(nc) as tc:
        # ---- phase 0a: fp32 replicate-padded images in DRAM (fixup path) ----
        for b in range(BPC):
            nc.sync.dma_start(out=pp3[b, PAD:PAD + H, PAD:PAD + W],
                              in_=img[b])
            for k in range(PAD):
                nc.sync.dma_start(out=pp3[b, k:k + 1, PAD:PAD + W],
                                  in_=img[b, 0:1, :])
                nc.sync.dma_start(
                    out=pp3[b, PAD + H + k:PAD + H + k + 1, PAD:PAD + W],
                    in_=img[b, H - 1:H, :])

        # column replication through SBUF (broadcast), avoiding the
        # 1-element-per-descriptor DRAM->DRAM column DMAs
        rblocks = []
        rs = 0
        while rs < PP:
            rn = min(128, PP - rs)
            rblocks.append((rs, rn))
            rs += rn

        with tc.tile_pool(name="pad", bufs=2) as pd:
            for b in range(BPC):
                for (rs, rn) in rblocks:
                    cl = pd.tile([128, 1], FP, tag="cl")
                    nc.sync.dma_start(out=cl[0:rn],
                                      in_=pp3[b, rs:rs + rn, PAD:PAD + 1])
                    ct = pd.tile([128, PAD], FP, tag="ct")
                    v.tensor_copy(out=ct[0:rn],
                                  in_=cl[0:rn, 0:1].broadcast_to((rn, PAD)))
                    nc.sync.dma_start(out=pp3[b, rs:rs + rn, 0:PAD],
                                      in_=ct[0:rn])
                    cr = pd.tile([128, 1], FP, tag="cr")
                    nc.sync.dma_start(
                        out=cr[0:rn],
                        in_=pp3[b, rs:rs + rn, PAD + W - 1:PAD + W])
                    cu = pd.tile([128, PAD], FP, tag="cu")
                    v.tensor_copy(out=cu[0:rn],
                                  in_=cr[0:rn, 0:1].broadcast_to((rn, PAD)))
                    nc.sync.dma_start(
                        out=pp3[b, rs:rs + rn, PAD + W:PAD + W + PAD],
                        in_=cu[0:rn])

        # ---- phase 0b: HP packed-pair fp16 image in DRAM ----
        # hp[b,h,c] = int32 packing of (fp16(ppad[b,h,c]), fp16(ppad[b,h,c+1]))
        with tc.tile_pool(name="hpb", bufs=2) as hb:
            for b in range(BPC):
                for (rs, rn) in rblocks:
                    p32 = hb.tile([128, PP], FP, tag="p32")
                    nc.sync.dma_start(out=p32[0:rn],
                                      in_=pp3[b, rs:rs + rn, :])
                    hpt = hb.tile([128, PP], I32, tag="hpt")
                    v16 = hpt[0:rn].bitcast(FP16).rearrange(
                        "p (c two) -> p c two", two=2)
                    v.tensor_copy(out=v16[:, 0:PP - 1, 0:1],
                                  in_=p32[0:rn, 0:PP - 1].unsqueeze(-1))
                    nc.scalar.copy(out=v16[:, 0:PP - 1, 1:2],
                                   in_=p32[0:rn, 1:PP].unsqueeze(-1))
                    nc.sync.dma_start(out=hp3[b, rs:rs + rn, 0:PP - 1],
                                      in_=hpt[0:rn, 0:PP - 1])

        # ---- dense tiles ----
        TS = [t for t in range(S_LO, S_HI + 1) if t != INIT]
        with tc.tile_pool(name="wk", bufs=2) as wk:
            for b in range(BPC):
                for i in range(NROW):
                    for j in range(NCOL):
                        r0 = 128 * i
                        w0 = F * j
                        hpt = []
                        for r in range(9):
                            t_h = wk.tile([128, F + 8], I32, tag=f"hp{r}")
                            nc.sync.dma_start(
                                out=t_h[:],
                                in_=hp3[b,
                                        PAD + r0 - 4 + r:PAD + r0 - 4 + r + 128,
                                        PAD + w0 - 4:PAD + w0 + F + 4])
                            hpt.append(t_h)
                        f32h = wk.tile([128, F], FP, tag="f32h")
                        nc.sync.dma_start(
                            out=f32h[:],
                            in_=flow4[b, 0, r0:r0 + 128, w0:w0 + F])
                        f32w = wk.tile([128, F], FP, tag="f32w")
                        nc.sync.dma_start(
                            out=f32w[:],
                            in_=flow4[b, 1, r0:r0 + 128, w0:w0 + F])
                        dh16 = wk.tile([128, F], FP16, tag="dh16")
                        nc.scalar.copy(out=dh16[:], in_=f32h[:])
                        dw16 = wk.tile([128, F], FP16, tag="dw16")
                        nc.scalar.copy(out=dw16[:], in_=f32w[:])

                        # integer parts: the DVE float->int convert ROUNDS to
                        # nearest, so bias by 7.5: round(x+7.5) = floor(x)+8
                        # (half-way ties only at integral x, where the lerp is
                        # continuous, so either neighbor gives the same value)
                        nw8 = wk.tile([128, F], I16, tag="nw8")
                        v.tensor_scalar(out=nw8[:], in0=dw16[:], scalar1=7.5,
                                        scalar2=None, op0=AL.add)
                        kh8 = wk.tile([128, F], I16, tag="kh8")
                        v.tensor_scalar(out=kh8[:], in0=dh16[:], scalar1=7.5,
                                        scalar2=None, op0=AL.add)

                        mw = {}
                        for t in TS:
                            m = wk.tile([128, F], I16, tag=f"mw{t}")
                            v.tensor_scalar(out=m[:], in0=nw8[:],
                                            scalar1=float(t + 8), scalar2=None,
                                            op0=AL.is_equal)
                            mw[t] = m
                        mv = {}
                        for s in TS:
                            m = wk.tile([128, F], I16, tag=f"mv{s}")
                            v.tensor_scalar(out=m[:], in0=kh8[:],
                                            scalar1=float(s + 8), scalar2=None,
                                            op0=AL.is_equal)
                            mv[s] = m

                        # fractions
                        flw = wk.tile([128, F], FP16, tag="flw")
                        v.tensor_scalar(out=flw[:], in0=nw8[:], scalar1=8.0,
                                        scalar2=None, op0=AL.subtract)
                        gam = wk.tile([128, F], FP16, tag="gam")
                        v.tensor_tensor(out=gam[:], in0=dw16[:], in1=flw[:],
                                        op=AL.subtract)
                        flh = wk.tile([128, F], FP16, tag="flh")
                        v.tensor_scalar(out=flh[:], in0=kh8[:], scalar1=8.0,
                                        scalar2=None, op0=AL.subtract)
                        alp = wk.tile([128, F], FP16, tag="alp")
                        v.tensor_tensor(out=alp[:], in0=dh16[:], in1=flh[:],
                                        op=AL.subtract)

                        # horizontal one-hot per candidate row (packed pairs)
                        C = []
                        for r in range(9):
                            c_t = wk.tile([128, F], I32, tag=f"c{r}")
                            v.tensor_copy(out=c_t[:],
                                          in_=hpt[r][:, INIT + 4:INIT + 4 + F])
                            for t in TS:
                                v.copy_predicated(
                                    out=c_t[:], mask=mw[t][:],
                                    data=hpt[r][:, t + 4:t + 4 + F])
                            C.append(c_t)

                        # vertical one-hot: N pair (row hf), S pair (row hf+1)
                        np_t = wk.tile([128, F], I32, tag="np")
                        v.tensor_copy(out=np_t[:], in_=C[INIT + 4][:])
                        for s in TS:
                            v.copy_predicated(out=np_t[:], mask=mv[s][:],
                                              data=C[s + 4][:])
                        sp_t = wk.tile([128, F], I32, tag="sp")
                        v.tensor_copy(out=sp_t[:], in_=C[INIT + 5][:])
                        for s in TS:
                            v.copy_predicated(out=sp_t[:], mask=mv[s][:],
                                              data=C[s + 5][:])

                        # unpack + bilinear lerp (fp16)
                        npv = np_t[:].bitcast(FP16).rearrange(
                            "p (c two) -> p c two", two=2)
                        spv = sp_t[:].bitcast(FP16).rearrange(
                            "p (c two) -> p c two", two=2)
                        nw_v = npv[:, :, 0:1].squeeze(-1)
                        ne_v = npv[:, :, 1:2].squeeze(-1)
                        sw_v = spv[:, :, 0:1].squeeze(-1)
                        se_v = spv[:, :, 1:2].squeeze(-1)

                        dn = wk.tile([128, F], FP16, tag="dn")
                        v.tensor_tensor(out=dn[:], in0=ne_v, in1=nw_v,
                                        op=AL.subtract)
                        t1 = wk.tile([128, F], FP16, tag="t1")
                        v.tensor_tensor(out=t1[:], in0=gam[:], in1=dn[:],
                                        op=AL.mult)
                        hn = wk.tile([128, F], FP16, tag="hn")
                        v.tensor_tensor(out=hn[:], in0=nw_v, in1=t1[:],
                                        op=AL.add)
                        ds = wk.tile([128, F], FP16, tag="ds")
                        v.tensor_tensor(out=ds[:], in0=se_v, in1=sw_v,
                                        op=AL.subtract)
                        t2 = wk.tile([128, F], FP16, tag="t2")
                        v.tensor_tensor(out=t2[:], in0=gam[:], in1=ds[:],
                                        op=AL.mult)
                        hs = wk.tile([128, F], FP16, tag="hs")
                        v.tensor_tensor(out=hs[:], in0=sw_v, in1=t2[:],
                                        op=AL.add)
                        dv_ = wk.tile([128, F], FP16, tag="dv")
                        v.tensor_tensor(out=dv_[:], in0=hs[:], in1=hn[:],
                                        op=AL.subtract)
                        t3 = wk.tile([128, F], FP16, tag="t3")
                        v.tensor_tensor(out=t3[:], in0=alp[:], in1=dv_[:],
                                        op=AL.mult)
                        o16 = wk.tile([128, F], FP16, tag="o16")
                        v.tensor_tensor(out=o16[:], in0=hn[:], in1=t3[:],
                                        op=AL.add)
                        nc.sync.dma_start(out=out3[b, r0:r0 + 128, w0:w0 + F],
                                          in_=o16[:])

        # ---- sparse fixup (exact fp32, as baseline) ----
        NCH = nout // 128
        with tc.tile_pool(name="fix", bufs=1) as fx:
            def load_aux(d, dt, name):
                t = fx.tile([128, NCH], dt, tag=name)
                nc.sync.dma_start(
                    out=t[:],
                    in_=d.ap().rearrange("(p f) -> p f", p=128))
                return t

            opos_s = load_aux(opos_d, I32, "opos")
            odh_s = load_aux(odh_d, I32, "odh")
            odw_s = load_aux(odw_d, I32, "odw")
            oh_s = load_aux(oh_d, FP, "oh")
            oh1_s = load_aux(oh1_d, FP, "oh1")
            ow_s = load_aux(ow_d, FP, "ow")
            ow1_s = load_aux(ow1_d, FP, "ow1")
            obase_s = load_aux(obase_d, FP, "obase")

            dhv = fx.tile([128, NCH], FP, tag="dhv")
            dwv = fx.tile([128, NCH], FP, tag="dwv")
            for c in range(NCH):
                g.indirect_dma_start(
                    out=dhv[:, c:c + 1], out_offset=None,
                    in_=flowf[:, None],
                    in_offset=IndirectOffsetOnAxis(
                        ap=odh_s[:, c:c + 1], axis=0))
                g.indirect_dma_start(
                    out=dwv[:, c:c + 1], out_offset=None,
                    in_=flowf[:, None],
                    in_offset=IndirectOffsetOnAxis(
                        ap=odw_s[:, c:c + 1], axis=0))

            def fields(dv, hb, hb1, pfx):
                yt = fx.tile([128, NCH], FP, tag=f"{pfx}y")
                v.tensor_tensor(out=yt[:], in0=dv[:], in1=hb[:],
                                op=AL.add)
                v.tensor_scalar(out=yt[:], in0=yt[:], scalar1=1.0,
                                scalar2=None, op0=AL.add)
                Rt = fx.tile([128, NCH], FP, tag=f"{pfx}R")
                v.tensor_tensor(out=Rt[:], in0=yt[:], in1=hb1[:],
                                op=AL.subtract)
                # floor over full range [-7, 7): floor(R) = S - 7
                St = fx.tile([128, NCH], FP, tag=f"{pfx}S")
                gt = fx.tile([128, NCH], FP, tag=f"{pfx}g")
                v.tensor_scalar(out=St[:], in0=Rt[:], scalar1=-6.0,
                                scalar2=None, op0=AL.is_ge)
                for s in range(-5, 7):
                    v.tensor_scalar(out=gt[:], in0=Rt[:], scalar1=float(s),
                                    scalar2=None, op0=AL.is_ge)
                    v.tensor_tensor(out=St[:], in0=St[:], in1=gt[:],
                                    op=AL.add)
                dt_ = fx.tile([128, NCH], FP, tag=f"{pfx}d")
                # d = (floor(R) + 1) - R = (S - 6) - R
                v.tensor_scalar(out=dt_[:], in0=St[:], scalar1=-6.0,
                                scalar2=None, op0=AL.add)
                v.tensor_tensor(out=dt_[:], in0=dt_[:], in1=Rt[:],
                                op=AL.subtract)
                return yt, dt_

            yv, dhw = fields(dhv, oh_s, oh1_s, "fh")
            ywv, dww = fields(dwv, ow_s, ow1_s, "fw")

            # addresses: a = ((y + dh) + 6)*PP + ((yw + dw) + 6) + base
            rowp = fx.tile([128, NCH], FP, tag="rowp")
            v.tensor_tensor(out=rowp[:], in0=yv[:], in1=dhw[:], op=AL.add)
            v.tensor_scalar(out=rowp[:], in0=rowp[:], scalar1=6.0,
                            scalar2=float(PP), op0=AL.add, op1=AL.mult)
            colp = fx.tile([128, NCH], FP, tag="colp")
            v.tensor_tensor(out=colp[:], in0=ywv[:], in1=dww[:],
                            op=AL.add)
            v.tensor_scalar(out=colp[:], in0=colp[:], scalar1=6.0,
                            scalar2=None, op0=AL.add)
            af = fx.tile([128, NCH], FP, tag="af")
            v.tensor_tensor(out=af[:], in0=rowp[:], in1=colp[:],
                            op=AL.add)
            v.tensor_tensor(out=af[:], in0=af[:], in1=obase_s[:],
                            op=AL.add)

            vals = {}
            afo = fx.tile([128, NCH], FP, tag="afo")
            for (cn, doff) in (("v00", 0.0), ("v10", 1.0),
                               ("v01", float(PP)), ("v11", float(PP + 1))):
                ai = fx.tile([128, NCH], I32, tag=f"ai{cn}")
                if doff == 0.0:
                    v.tensor_copy(out=ai[:], in_=af[:])
                else:
                    v.tensor_scalar(out=afo[:], in0=af[:], scalar1=doff,
                                    scalar2=None, op0=AL.add)
                    v.tensor_copy(out=ai[:], in_=afo[:])
                vt = fx.tile([128, NCH], FP, tag=cn)
                for c in range(NCH):
                    g.indirect_dma_start(
                        out=vt[:, c:c + 1], out_offset=None,
                        in_=ppf[:, None],
                        in_offset=IndirectOffsetOnAxis(
                            ap=ai[:, c:c + 1], axis=0))
                vals[cn] = vt

            omw_f = fx.tile([128, NCH], FP, tag="omwf")
            v.tensor_scalar(out=omw_f[:], in0=dww[:], scalar1=-1.0,
                            scalar2=1.0, op0=AL.mult, op1=AL.add)
            omh_f = fx.tile([128, NCH], FP, tag="omhf")
            v.tensor_scalar(out=omh_f[:], in0=dhw[:], scalar1=-1.0,
                            scalar2=1.0, op0=AL.mult, op1=AL.add)
            wt = fx.tile([128, NCH], FP, tag="wtf")
            accf = fx.tile([128, NCH], FP, tag="accf")
            t3f = fx.tile([128, NCH], FP, tag="t3f")
            v.tensor_tensor(out=wt[:], in0=dhw[:], in1=dww[:], op=AL.mult)
            v.tensor_tensor(out=accf[:], in0=vals["v00"][:], in1=wt[:],
                            op=AL.mult)
            v.tensor_tensor(out=wt[:], in0=dhw[:], in1=omw_f[:],
                            op=AL.mult)
            v.tensor_tensor(out=t3f[:], in0=vals["v10"][:], in1=wt[:],
                            op=AL.mult)
            v.tensor_tensor(out=accf[:], in0=accf[:], in1=t3f[:],
                            op=AL.add)
            v.tensor_tensor(out=wt[:], in0=omh_f[:], in1=dww[:],
                            op=AL.mult)
            v.tensor_tensor(out=t3f[:], in0=vals["v01"][:], in1=wt[:],
                            op=AL.mult)
            v.tensor_tensor(out=accf[:], in0=accf[:], in1=t3f[:],
                            op=AL.add)
            v.tensor_tensor(out=wt[:], in0=omw_f[:], in1=omh_f[:],
                            op=AL.mult)
            v.tensor_tensor(out=t3f[:], in0=vals["v11"][:], in1=wt[:],
                            op=AL.mult)
            v.tensor_tensor(out=accf[:], in0=accf[:], in1=t3f[:],
                            op=AL.add)

            acc16 = fx.tile([128, NCH], FP16, tag="acc16")
            v.tensor_copy(out=acc16[:], in_=accf[:])
            for c in range(NCH):
                g.indirect_dma_start(
                    out=outf[:, None],
                    out_offset=IndirectOffsetOnAxis(
                        ap=opos_s[:, c:c + 1], axis=0),
                    in_=acc16[:, c:c + 1], in_offset=None)

    nc.compile()
    return nc


_PROGRAM_CACHE = {}


def _get_program(nout):
    if nout not in _PROGRAM_CACHE:
        _PROGRAM_CACHE[nout] = _build_program(nout)
    return _PROGRAM_CACHE[nout]


def _host_inlier_mask(d):
    """Mirror the device fp16 floor: rint(fp32(fp16(d)) + 7.5) in [4, 11]."""
    d16 = d.astype(np.float16).astype(np.float32)
    n8 = np.rint(d16 + np.float32(7.5)).astype(np.int32)
    return (n8 >= S_LO + 8) & (n8 <= S_HI + 8)


def _host_metadata(dH, dW):
    """Outlier positions for one image under the device dense criterion."""
    inl = _host_inlier_mask(dH) & _host_inlier_mask(dW)
    oy, ox = np.where(~inl)
    return oy.astype(np.int64), ox.astype(np.int64)


def _prepare(input1, input2):
    """Build (or fetch) the program and the per-core input maps."""
    input1 = np.asarray(input1)
    input2 = np.asarray(input2)
    assert input1.shape == (B, 1, H, W) and input2.shape == (B, 2, H, W)

    # per-core host metadata
    metas = []
    max_n = 1
    for c in range(NCORES):
        rows = []
        for bl in range(BPC):
            bglob = c * BPC + bl
            oy, ox = _host_metadata(input2[bglob, 0], input2[bglob, 1])
            rows.append((bl, oy, ox))
        n = sum(len(oy) for _, oy, _ in rows)
        max_n = max(max_n, n)
        metas.append(rows)
    nout = max(128, ((max_n + 127) // 128) * 128)

    nc = _get_program(nout)

    in_maps = []
    for c in range(NCORES):
        imgs = input1[c * BPC:(c + 1) * BPC, 0]
        flow = input2[c * BPC:(c + 1) * BPC]
        opos = np.full(nout, BPC * HW, np.int32)
        odh = np.zeros(nout, np.int32)
        odw = np.full(nout, HW, np.int32)
        oh = np.zeros(nout, f32)
        ow = np.zeros(nout, f32)
        obase = np.zeros(nout, f32)
        k = 0
        for bl, oy, ox in metas[c]:
            n = len(oy)
            opos[k:k + n] = (bl * HW + oy * W + ox).astype(np.int32)
            odh[k:k + n] = (bl * 2 * HW + oy * W + ox).astype(np.int32)
            odw[k:k + n] = (bl * 2 * HW + HW + oy * W + ox).astype(np.int32)
            oh[k:k + n] = oy.astype(f32)
            ow[k:k + n] = ox.astype(f32)
            obase[k:k + n] = f32(bl * PP * PP)
            k += n
        in_maps.append({
            "img": np.ascontiguousarray(imgs),
            "flow": np.ascontiguousarray(flow.reshape(-1)),
            "opos": opos, "odh": odh, "odw": odw,
            "oh": oh, "oh1": (oh + f32(1.0)).astype(f32),
            "ow": ow, "ow1": (ow + f32(1.0)).astype(f32),
            "obase": obase,
        })

    return nc, in_maps


def _assemble(results):
    out = np.empty((B, 1, H, W), f32)
    for c in range(NCORES):
        o = results[c]["out"][:BPC * HW].reshape(BPC, H, W)
        out[c * BPC:(c + 1) * BPC, 0] = o.astype(f32)
    return out


def kernel(input1, input2):
    nc, in_maps = _prepare(input1, input2)
    res = run_bass_kernel_spmd(nc, in_maps, core_ids=list(range(NCORES)))
    return _assemble(res.results)
